# revision 7
# baseline (speedup 1.0000x reference)
"""Distributed Trainium2 kernel for GNN message passing (bilinear pooling GCN layer).

out = diag * 0.5 * ((A@Y)^2 - A@(Y^2)),  Y = feat @ W.T + b
A: sparse COO [100000, 100000], 1.6M edges.

Strategy (8 NeuronCores, row-parallel SpMM):
  - core c owns global rows [c*12500, (c+1)*12500): computes its Y shard
    on-device (PE transpose + matmul), AllGather -> full Y table in DRAM.
  - edges pre-sorted on host by (owner core, 64-row window); each window's
    edges packed into 128-edge tiles, tile count equalized across cores so
    all 8 cores run one identical (SPMD) graph.
  - per edge tile: indirect-DMA gather Y[col] -> G [128,64]; Act square ->
    G2; DVE/Pool one-op one-hot build S'[p,j] = (j==rowoff[p])*val[p]
    (tensor_scalar is_equal+mult vs an iota constant); PE matmul
    psum[64,128] += S'^T @ [G|G2] accumulated over the window.
  - window epilogue: s=psum[:,:64], sq=psum[:,64:]:
    out_rows = (0.5*diag) * (s*s - sq); contiguous DMA to out shard.
"""
import sys

sys.path.insert(0, "/opt/trn_rl_repo")

import numpy as np

N = 100000
NC = 8
NLOC = N // NC          # 12500
D = 64
DIN = 256
R = 64                  # window rows (one-hot width / psum partitions)
NW = (NLOC + R - 1) // R  # 196
P = 128                 # edge tile partition size
NIT = (NLOC + P - 1) // P  # 98 feat tiles


# ----------------------------------------------------------------- host prep
def preprocess(adj_rows, adj_cols, adj_vals):
    """Pack edges into per-core [NW, 128, TMAX] slabs (partition-major)."""
    owner = adj_rows // NLOC
    lr = adj_rows - owner * NLOC
    w = lr // R
    rowoff = (lr - w * R).astype(np.float32)

    cnt = np.zeros((NC, NW), dtype=np.int64)
    np.add.at(cnt, (owner, w), 1)
    tmax = int(np.maximum(1, (cnt + P - 1) // P).max())

    per_core = []
    for c in range(NC):
        m = owner == c
        cw, ccol, cval, croff = w[m], adj_cols[m], adj_vals[m], rowoff[m]
        order = np.argsort(cw, kind="stable")
        cw, ccol, cval, croff = cw[order], ccol[order], cval[order], croff[order]
        start_of_w = np.searchsorted(cw, np.arange(NW))
        slot = np.arange(len(cw)) - start_of_w[cw]
        t_idx, p_idx = slot // P, slot % P
        flat = cw * (P * tmax) + p_idx * tmax + t_idx   # [NW,128,TMAX] layout
        cols = np.zeros(NW * P * tmax, dtype=np.int32)
        vals = np.zeros(NW * P * tmax, dtype=np.float32)
        roff = np.zeros(NW * P * tmax, dtype=np.float32)
        cols[flat] = ccol
        vals[flat] = cval
        roff[flat] = croff
        per_core.append((cols.reshape(NW, P, tmax),
                         vals.reshape(NW, P, tmax),
                         roff.reshape(NW, P, tmax)))
    return tmax, per_core


# --------------------------------------------------------------- graph build
def build(tmax):
    from concourse import bacc, bass, mybir, tile
    from concourse.masks import make_identity

    f32 = mybir.dt.float32
    i32 = mybir.dt.int32
    nc = bacc.Bacc("TRN2", target_bir_lowering=False, debug=False,
                   num_devices=NC)

    feat = nc.dram_tensor("feat", [NLOC, DIN], f32, kind="ExternalInput")
    w_in = nc.dram_tensor("W", [D, DIN], f32, kind="ExternalInput")
    b_in = nc.dram_tensor("b", [1, D], f32, kind="ExternalInput")
    cols = nc.dram_tensor("cols", [NW, P, tmax], i32, kind="ExternalInput")
    vals = nc.dram_tensor("vals", [NW, P, tmax], f32, kind="ExternalInput")
    roff = nc.dram_tensor("roff", [NW, P, tmax], f32, kind="ExternalInput")
    dhalf = nc.dram_tensor("dhalf", [NW * R], f32, kind="ExternalInput")
    iota_in = nc.dram_tensor("iota", [P, R], f32, kind="ExternalInput")
    out = nc.dram_tensor("out", [NLOC, D], f32, kind="ExternalOutput")

    yb = nc.dram_tensor("yb", [NLOC, D], f32)
    yfull = nc.dram_tensor("yfull", [N, D], f32, addr_space="Shared")

    with tile.TileContext(nc) as tc:
        with (
            tc.tile_pool(name="const", bufs=1) as cp,
            tc.tile_pool(name="sbxw", bufs=3) as sbxw,
            tc.tile_pool(name="sbw", bufs=2) as sbw,
            tc.tile_pool(name="sbt", bufs=4) as sbt,
            tc.tile_pool(name="sbe", bufs=2) as sbe,
            tc.tile_pool(name="ps", bufs=2, space="PSUM") as ps,
        ):
            # ---- constants
            ident = cp.tile([P, P], f32)
            make_identity(nc, ident[:])
            iota_sb = cp.tile([P, R], f32)
            nc.sync.dma_start(out=iota_sb[:], in_=iota_in[:, :])
            ones_sb = cp.tile([1, P], f32)
            nc.vector.memset(ones_sb[:], 1.0)
            b_sb = cp.tile([1, D], f32)
            nc.sync.dma_start(out=b_sb[:], in_=b_in[:, :])
            # W [64,256] -> WT chunks [128, 2*64]
            w_raw = cp.tile([D, DIN], f32)
            nc.sync.dma_start(out=w_raw[:], in_=w_in[:, :])
            wt_sb = cp.tile([P, 2 * D], f32)
            for ch in range(2):
                pt = ps.tile([P, D], f32, tag="wtp")
                nc.tensor.transpose(out=pt[:], in_=w_raw[:, ch * P:(ch + 1) * P],
                                    identity=ident[:D, :D])
                nc.vector.tensor_copy(out=wt_sb[:, ch * D:(ch + 1) * D], in_=pt[:])

            # ---- phase 1: Y = feat @ W.T + b  (shard), DMA to yb
            for it in range(NIT):
                mi = min(P, NLOC - it * P)
                ft = sbxw.tile([P, DIN], f32, tag="ft")
                nc.sync.dma_start(out=ft[:mi], in_=feat[it * P:it * P + mi, :])
                xt = sbxw.tile([P, DIN], f32, tag="xt")
                for ch in range(2):
                    pt = ps.tile([P, P], f32, tag="tp")
                    nc.tensor.transpose(out=pt[:, :mi],
                                        in_=ft[:mi, ch * P:(ch + 1) * P],
                                        identity=ident[:mi, :mi])
                    nc.vector.tensor_copy(out=xt[:, ch * P:ch * P + mi],
                                          in_=pt[:, :mi])
                py = ps.tile([P, D], f32, tag="py")
                nc.tensor.matmul(out=py[:mi], lhsT=xt[:, 0:mi],
                                 rhs=wt_sb[:, 0:D], start=True, stop=False)
                nc.tensor.matmul(out=py[:mi], lhsT=xt[:, P:P + mi],
                                 rhs=wt_sb[:, D:2 * D], start=False, stop=False)
                nc.tensor.matmul(out=py[:mi], lhsT=ones_sb[:, :mi],
                                 rhs=b_sb[:, :], start=False, stop=True)
                ysb = sbxw.tile([P, D], f32, tag="ysb")
                nc.scalar.copy(out=ysb[:mi], in_=py[:mi])
                nc.sync.dma_start(out=yb[it * P:it * P + mi, :], in_=ysb[:mi])

            # ---- allgather Y
            from concourse.bass import IndirectOffsetOnAxis
            nc.gpsimd.collective_compute(
                "AllGather",
                mybir.AluOpType.bypass,
                replica_groups=[list(range(NC))],
                ins=[yb.ap().opt()],
                outs=[yfull.ap().opt()],
            )

            # ---- phase 2: edge processing
            for wi in range(NW):
                rows = min(R, NLOC - wi * R)
                roff_sb = sbw.tile([P, tmax], f32, tag="roff")
                vals_sb = sbw.tile([P, tmax], f32, tag="vals")
                cols_sb = sbw.tile([P, tmax], i32, tag="cols")
                nc.sync.dma_start(out=roff_sb[:], in_=roff[wi])
                nc.sync.dma_start(out=vals_sb[:], in_=vals[wi])
                nc.sync.dma_start(out=cols_sb[:], in_=cols[wi])
                pw = ps.tile([R, 2 * D], f32, tag="pw")
                for t in range(tmax):
                    rhs = sbt.tile([P, 2 * D], f32, tag="rhs")
                    nc.gpsimd.indirect_dma_start(
                        out=rhs[:, :D], out_offset=None,
                        in_=yfull[:, :],
                        in_offset=IndirectOffsetOnAxis(ap=cols_sb[:, t:t + 1],
                                                       axis=0),
                    )
                    nc.scalar.square(out=rhs[:, D:], in_=rhs[:, :D])
                    sp = sbt.tile([P, R], f32, tag="sp")
                    eng = nc.vector if t % 2 == 0 else nc.gpsimd
                    eng.tensor_scalar(
                        out=sp[:], in0=iota_sb[:],
                        scalar1=roff_sb[:, t:t + 1], scalar2=vals_sb[:, t:t + 1],
                        op0=mybir.AluOpType.is_equal, op1=mybir.AluOpType.mult,
                    )
                    nc.tensor.matmul(out=pw[:], lhsT=sp[:], rhs=rhs[:],
                                     start=(t == 0), stop=(t == tmax - 1))
                # epilogue
                s2 = sbe.tile([R, D], f32, tag="s2")
                nc.scalar.square(out=s2[:], in_=pw[:, :D])
                dsb = sbe.tile([R, D], f32, tag="d")
                nc.vector.tensor_sub(dsb[:], s2[:], pw[:, D:])
                dg = sbe.tile([R, 1], f32, tag="dg")
                nc.sync.dma_start(out=dg[:], in_=dhalf[wi * R:wi * R + R, None])
                osb = sbe.tile([R, D], f32, tag="osb")
                nc.scalar.mul(out=osb[:], in_=dsb[:], mul=dg[:])
                nc.sync.dma_start(out=out[wi * R:wi * R + rows, :],
                                  in_=osb[:rows])
    nc.compile()
    return nc


# --------------------------------------------------------------------- entry
def kernel(feat, W, b, adj_vals, diag_vals, adj_rows, adj_cols):
    from concourse.bass_utils import run_bass_kernel_spmd

    feat = np.ascontiguousarray(np.asarray(feat, dtype=np.float32))
    W = np.ascontiguousarray(np.asarray(W, dtype=np.float32))
    b = np.asarray(b, dtype=np.float32)
    adj_vals = np.asarray(adj_vals, dtype=np.float32)
    diag_vals = np.asarray(diag_vals, dtype=np.float32)
    adj_rows = np.asarray(adj_rows, dtype=np.int32)
    adj_cols = np.asarray(adj_cols, dtype=np.int32)

    tmax, per_core = preprocess(adj_rows, adj_cols, adj_vals)
    iota = np.tile(np.arange(R, dtype=np.float32), (P, 1))

    in_maps = []
    for c in range(NC):
        ccols, cvals, croff = per_core[c]
        dh = np.zeros(NW * R, dtype=np.float32)
        dh[:NLOC] = 0.5 * diag_vals[c * NLOC:(c + 1) * NLOC]
        in_maps.append({
            "feat": feat[c * NLOC:(c + 1) * NLOC],
            "W": W,
            "b": b.reshape(1, D),
            "cols": ccols,
            "vals": cvals,
            "roff": croff,
            "dhalf": dh,
            "iota": iota,
        })

    nc = build(tmax)
    import os
    trace = bool(int(os.environ.get("KERNEL_TRACE", "0")))
    res = run_bass_kernel_spmd(nc, in_maps, core_ids=list(range(NC)),
                               trace=trace)
    global LAST_EXEC_NS, LAST_RESULT
    LAST_EXEC_NS = res.exec_time_ns
    LAST_RESULT = res
    return np.concatenate([res.results[c]["out"] for c in range(NC)], axis=0)
